# revision 12
# baseline (speedup 1.0000x reference)
"""Multi-head SwiGLU feed-forward (nn_MultiHeadFeedForward) Trainium2 kernel.

Math (per head h of 16, head_dim d=128, ffn f=512):
    g = x_h @ gate_w[h]      # [T,128]@[128,512]
    u = x_h @ up_w[h]
    out_h = (silu(g)*u) @ down_w[h]   # [T,512]@[512,128]

Sharding: 2 heads per core across 8 cores (no cross-core communication).
On-chip layout is feature-major ("transposed"): activations live as
[feature, token] tiles so every matmul contracts along the partition dim
without any on-chip transposes.  The host pre-transposes x into
xT[h, d, t] per core and un-transposes the output.
"""

import os
import sys

import numpy as np

for _p in ("/opt/trn_rl_repo",):
    if _p not in sys.path and os.path.isdir(_p):
        sys.path.insert(0, _p)

import concourse.bass as bass
import concourse.mybir as mybir
from concourse import bacc
import concourse.tile as tile
from concourse.bass_utils import run_bass_kernel_spmd

B, S, EMB = 4, 4096, 2048
HEADS, HD, FFN = 16, 128, 512
T = B * S                      # 16384 tokens
N_CORES = 8
HPC = HEADS // N_CORES         # heads per core = 2
TOK = 256                      # tokens per on-chip tile
NT = T // TOK                  # token tiles per head
NCH = FFN // HD                # ffn chunks of 128 = 4
SLAB = 4096                    # output slab tokens (drained by 2 half DMAs)

F32 = mybir.dt.float32
BF16 = mybir.dt.bfloat16
AF = mybir.ActivationFunctionType


def _build_nc():
    nc = bacc.Bacc("TRN2", target_bir_lowering=False)

    xT = nc.dram_tensor("xT", [HPC, HD, T], BF16, kind="ExternalInput")
    gw = nc.dram_tensor("gw", [HPC, HD, FFN], BF16, kind="ExternalInput")
    uw = nc.dram_tensor("uw", [HPC, HD, FFN], BF16, kind="ExternalInput")
    dw = nc.dram_tensor("dw", [HPC, FFN, HD], BF16, kind="ExternalInput")
    outT = nc.dram_tensor("outT", [HPC, HD, T], F32, kind="ExternalOutput")

    # Output accumulates in SBUF slabs of SLAB tokens, drained by two large
    # read-only DMAs each (one per copy-engine half) so every DMA needs at
    # most ONE semaphore wait (walrus DIRECT2D DMAs only support one).
    TPS = SLAB // TOK  # tiles per slab

    with tile.TileContext(nc) as tc:
        with (
            tc.tile_pool(name="wpool", bufs=1) as wpool,
            tc.tile_pool(name="gpool", bufs=2, space="PSUM") as gpool,
            tc.tile_pool(name="upool", bufs=1, space="PSUM") as upool,
            tc.tile_pool(name="opool", bufs=2, space="PSUM") as opool,
            tc.tile_pool(name="sgpool", bufs=3) as sgpool,
            tc.tile_pool(name="hpool", bufs=3) as hpool,
            tc.tile_pool(name="slabs", bufs=2) as slabs,
        ):
            # weights + the entire x shard resident in SBUF for the kernel
            gw_s = wpool.tile([HD, HPC, FFN], BF16)
            uw_s = wpool.tile([HD, HPC, FFN], BF16)
            dw_s = wpool.tile([HD, HPC, NCH, HD], BF16)
            xs_full = wpool.tile([HD, HPC, T], BF16)
            for h in range(HPC):
                nc.sync.dma_start(out=gw_s[:, h, :], in_=gw[h])
                nc.sync.dma_start(out=uw_s[:, h, :], in_=uw[h])
                nc.sync.dma_start(
                    out=dw_s[:, h, :, :],
                    in_=dw[h].rearrange("(c p) d -> p c d", p=HD),
                )
                for xc in range(8):  # write-once chunked loads (pipeline-able)
                    c0 = xc * (T // 8)
                    nc.sync.dma_start(
                        out=xs_full[:, h, c0 : c0 + T // 8],
                        in_=xT[h, :, c0 : c0 + T // 8],
                    )

            for h in range(HPC):
                for t in range(NT):
                    t0 = t * TOK
                    ts = t % TPS  # tile index within slab
                    if ts == 0:
                        slab = slabs.tile([HD, SLAB], F32, name=f"slab_{h}_{t}", tag="slab")
                    xs = xs_full[:, h, t0 : t0 + TOK]

                    gps = gpool.tile([HD, NCH * TOK], F32)
                    for c in range(NCH):
                        nc.tensor.matmul(
                            gps[:, c * TOK : (c + 1) * TOK],
                            lhsT=gw_s[:, h, c * HD : (c + 1) * HD],
                            rhs=xs,
                            start=True,
                            stop=True,
                        )
                    ups = upool.tile([HD, NCH * TOK], F32)
                    for c in range(NCH):
                        nc.tensor.matmul(
                            ups[:, c * TOK : (c + 1) * TOK],
                            lhsT=uw_s[:, h, c * HD : (c + 1) * HD],
                            rhs=xs,
                            start=True,
                            stop=True,
                        )

                    sg = sgpool.tile([HD, NCH * TOK], BF16)
                    nc.scalar.activation(sg[:], gps[:], AF.Silu)
                    hh = hpool.tile([HD, NCH * TOK], BF16)
                    nc.vector.tensor_mul(hh[:], sg[:], ups[:])

                    ops = opool.tile([HD, TOK], F32)
                    for c in range(NCH):
                        nc.tensor.matmul(
                            ops[:],
                            lhsT=dw_s[:, h, c, :],
                            rhs=hh[:, c * TOK : (c + 1) * TOK],
                            start=(c == 0),
                            stop=(c == NCH - 1),
                        )

                    # psum -> slab; first half of each slab via ScalarE, second
                    # via VectorE, so each half-drain DMA waits on one engine.
                    dst = slab[:, ts * TOK : (ts + 1) * TOK]
                    if ts < TPS // 2:
                        nc.scalar.copy(dst, ops[:])
                    else:
                        nc.vector.tensor_copy(dst, ops[:])

                    if ts == TPS // 2 - 1:  # ScalarE half complete
                        nc.sync.dma_start(
                            out=outT[h, :, t0 + TOK - SLAB // 2 : t0 + TOK],
                            in_=slab[:, : SLAB // 2],
                        )
                    elif ts == TPS - 1:  # VectorE half complete
                        nc.sync.dma_start(
                            out=outT[h, :, t0 + TOK - SLAB // 2 : t0 + TOK],
                            in_=slab[:, SLAB // 2 :],
                        )
    nc.compile()
    return nc


def _shard_inputs(inputs):
    import ml_dtypes

    bf16 = ml_dtypes.bfloat16
    x = np.asarray(inputs["x"], dtype=np.float32)
    gw = np.asarray(inputs["gate_w"], dtype=np.float32).astype(bf16)
    uw = np.asarray(inputs["up_w"], dtype=np.float32).astype(bf16)
    dw = np.asarray(inputs["down_w"], dtype=np.float32).astype(bf16)

    xh = x.reshape(T, HEADS, HD)
    xt = np.ascontiguousarray(xh.transpose(1, 2, 0)).astype(bf16)  # [16, 128, T]

    in_maps = []
    for c in range(N_CORES):
        hs = slice(HPC * c, HPC * (c + 1))
        in_maps.append(
            {
                "xT": xt[hs],
                "gw": gw[hs],
                "uw": uw[hs],
                "dw": dw[hs],
            }
        )
    return in_maps


def run(inputs, trace=False, **spmd_kwargs):
    nc = _build_nc()
    in_maps = _shard_inputs(inputs)
    res = run_bass_kernel_spmd(
        nc, in_maps, core_ids=list(range(N_CORES)), trace=trace, **spmd_kwargs
    )
    outT = np.empty((HEADS, HD, T), dtype=np.float32)
    for c in range(N_CORES):
        outT[HPC * c : HPC * (c + 1)] = res.results[c]["outT"]
    out = np.ascontiguousarray(outT.transpose(2, 0, 1)).reshape(B, S, EMB)
    return out, res


def kernel(**inputs):
    out, _ = run(inputs)
    return out


# revision 13
# speedup vs baseline: 1.2820x; 1.2820x over previous
"""Multi-head SwiGLU feed-forward (nn_MultiHeadFeedForward) Trainium2 kernel.

Math (per head h of 16, head_dim d=128, ffn f=512):
    g = x_h @ gate_w[h]      # [T,128]@[128,512]
    u = x_h @ up_w[h]
    out_h = (silu(g)*u) @ down_w[h]   # [T,512]@[512,128]

Sharding: 2 heads per core across 8 cores (no cross-core communication).
On-chip layout is feature-major ("transposed"): activations live as
[feature, token] tiles so every matmul contracts along the partition dim
without any on-chip transposes.  The host pre-transposes x into
xT[h, d, t] per core and un-transposes the output.
"""

import os
import sys

import numpy as np

for _p in ("/opt/trn_rl_repo",):
    if _p not in sys.path and os.path.isdir(_p):
        sys.path.insert(0, _p)

import concourse.bass as bass
import concourse.mybir as mybir
from concourse import bacc
import concourse.tile as tile
from concourse.bass_utils import run_bass_kernel_spmd

B, S, EMB = 4, 4096, 2048
HEADS, HD, FFN = 16, 128, 512
T = B * S                      # 16384 tokens
N_CORES = 8
HPC = HEADS // N_CORES         # heads per core = 2
TOK = 256                      # tokens per on-chip tile
NT = T // TOK                  # token tiles per head
NCH = FFN // HD                # ffn chunks of 128 = 4
SLAB = 4096                    # output slab tokens (drained by 2 half DMAs)

F32 = mybir.dt.float32
BF16 = mybir.dt.bfloat16
AF = mybir.ActivationFunctionType


def _build_nc():
    nc = bacc.Bacc("TRN2", target_bir_lowering=False)

    xT = nc.dram_tensor("xT", [HPC, HD, T], BF16, kind="ExternalInput")
    gw = nc.dram_tensor("gw", [HPC, HD, FFN], BF16, kind="ExternalInput")
    uw = nc.dram_tensor("uw", [HPC, HD, FFN], BF16, kind="ExternalInput")
    dw = nc.dram_tensor("dw", [HPC, FFN, HD], BF16, kind="ExternalInput")
    outT = nc.dram_tensor("outT", [HPC, HD, T], F32, kind="ExternalOutput")

    # Output accumulates in SBUF slabs of SLAB tokens, drained by two large
    # read-only DMAs each (one per copy-engine half) so every DMA needs at
    # most ONE semaphore wait (walrus DIRECT2D DMAs only support one).
    TPS = SLAB // TOK  # tiles per slab

    with tile.TileContext(nc) as tc:
        with (
            tc.tile_pool(name="wpool", bufs=1) as wpool,
            tc.tile_pool(name="gpool", bufs=2, space="PSUM") as gpool,
            tc.tile_pool(name="upool", bufs=2, space="PSUM") as upool,
            tc.tile_pool(name="sgpool", bufs=3) as sgpool,
            tc.tile_pool(name="hpool", bufs=3) as hpool,
            tc.tile_pool(name="slabs", bufs=2) as slabs,
        ):
            # weights + the entire x shard resident in SBUF for the kernel
            gw_s = wpool.tile([HD, HPC, FFN], BF16)
            uw_s = wpool.tile([HD, HPC, FFN], BF16)
            dw_s = wpool.tile([HD, HPC, NCH, HD], BF16)
            xs_full = wpool.tile([HD, HPC, T], BF16)
            for h in range(HPC):
                nc.sync.dma_start(out=gw_s[:, h, :], in_=gw[h])
                nc.sync.dma_start(out=uw_s[:, h, :], in_=uw[h])
                nc.sync.dma_start(
                    out=dw_s[:, h, :, :],
                    in_=dw[h].rearrange("(c p) d -> p c d", p=HD),
                )
                for xc in range(8):  # write-once chunked loads (pipeline-able)
                    c0 = xc * (T // 8)
                    nc.sync.dma_start(
                        out=xs_full[:, h, c0 : c0 + T // 8],
                        in_=xT[h, :, c0 : c0 + T // 8],
                    )

            # Software pipeline: tile k's down-proj + slab copy are emitted in
            # iteration k+1 so the PE never queues a stalled matmul ahead of
            # ready gate/up work.  The down-proj PSUM output is overlaid into
            # tile k's up-proj banks (free after the mul read them), keeping
            # total PSUM usage at 8 banks with everything double-buffered.
            slab = None
            prev = None  # (hh, ups, slab, h, t) of the previous tile

            def emit_down(p):
                phh, pups, pslab, ph, pt = p
                ops = pups[:, :TOK]
                for c in range(NCH):
                    nc.tensor.matmul(
                        ops,
                        lhsT=dw_s[:, ph, c, :],
                        rhs=phh[:, c * TOK : (c + 1) * TOK],
                        start=(c == 0),
                        stop=(c == NCH - 1),
                    )
                # psum -> slab; first half of each slab via ScalarE, second
                # via VectorE, so each half-drain DMA waits on one engine.
                pts = pt % TPS
                dst = pslab[:, pts * TOK : (pts + 1) * TOK]
                if pts < TPS // 2:
                    nc.scalar.copy(dst, ops)
                else:
                    nc.vector.tensor_copy(dst, ops)
                pt0 = pt * TOK
                if pts == TPS // 2 - 1:  # ScalarE half complete
                    nc.sync.dma_start(
                        out=outT[ph, :, pt0 + TOK - SLAB // 2 : pt0 + TOK],
                        in_=pslab[:, : SLAB // 2],
                    )
                elif pts == TPS - 1:  # VectorE half complete
                    nc.sync.dma_start(
                        out=outT[ph, :, pt0 + TOK - SLAB // 2 : pt0 + TOK],
                        in_=pslab[:, SLAB // 2 :],
                    )

            for h in range(HPC):
                for t in range(NT):
                    t0 = t * TOK
                    if t % TPS == 0:
                        slab = slabs.tile(
                            [HD, SLAB], F32, name=f"slab_{h}_{t}", tag="slab"
                        )
                    xs = xs_full[:, h, t0 : t0 + TOK]

                    gps = gpool.tile([HD, NCH * TOK], F32)
                    for c in range(NCH):
                        nc.tensor.matmul(
                            gps[:, c * TOK : (c + 1) * TOK],
                            lhsT=gw_s[:, h, c * HD : (c + 1) * HD],
                            rhs=xs,
                            start=True,
                            stop=True,
                        )
                    sg = sgpool.tile([HD, NCH * TOK], BF16)
                    nc.scalar.activation(sg[:], gps[:], AF.Silu)

                    ups = upool.tile([HD, NCH * TOK], F32)
                    for c in range(NCH):
                        nc.tensor.matmul(
                            ups[:, c * TOK : (c + 1) * TOK],
                            lhsT=uw_s[:, h, c * HD : (c + 1) * HD],
                            rhs=xs,
                            start=True,
                            stop=True,
                        )
                    hh = hpool.tile([HD, NCH * TOK], BF16)
                    nc.vector.tensor_mul(hh[:], sg[:], ups[:])

                    if prev is not None:
                        emit_down(prev)
                    prev = (hh, ups, slab, h, t)
            emit_down(prev)
    nc.compile()
    return nc


def _shard_inputs(inputs):
    import ml_dtypes

    bf16 = ml_dtypes.bfloat16
    x = np.asarray(inputs["x"], dtype=np.float32)
    gw = np.asarray(inputs["gate_w"], dtype=np.float32).astype(bf16)
    uw = np.asarray(inputs["up_w"], dtype=np.float32).astype(bf16)
    dw = np.asarray(inputs["down_w"], dtype=np.float32).astype(bf16)

    xh = x.reshape(T, HEADS, HD)
    xt = np.ascontiguousarray(xh.transpose(1, 2, 0)).astype(bf16)  # [16, 128, T]

    in_maps = []
    for c in range(N_CORES):
        hs = slice(HPC * c, HPC * (c + 1))
        in_maps.append(
            {
                "xT": xt[hs],
                "gw": gw[hs],
                "uw": uw[hs],
                "dw": dw[hs],
            }
        )
    return in_maps


def run(inputs, trace=False, **spmd_kwargs):
    nc = _build_nc()
    in_maps = _shard_inputs(inputs)
    res = run_bass_kernel_spmd(
        nc, in_maps, core_ids=list(range(N_CORES)), trace=trace, **spmd_kwargs
    )
    outT = np.empty((HEADS, HD, T), dtype=np.float32)
    for c in range(N_CORES):
        outT[HPC * c : HPC * (c + 1)] = res.results[c]["outT"]
    out = np.ascontiguousarray(outT.transpose(2, 0, 1)).reshape(B, S, EMB)
    return out, res


def kernel(**inputs):
    out, _ = run(inputs)
    return out
